# revision 30
# baseline (speedup 1.0000x reference)
"""Trainium2 Bass kernel for nn_Luban7_29609504539316 (BiLSTM + span pool + log_softmax).

Sharding (8 cores):
  - Direction-split scan: cores 0-3 run the FORWARD LSTM, cores 4-7 the BACKWARD
    LSTM (fed host-reversed tokens).  Core c handles batch group g = c % 4
    (batches g*8 .. g*8+8) for the scan.
  - Pair (c, c+4) exchanges hidden states (bf16, two time-chunked AllGathers
    issued mid-scan so the collective overlaps the remaining scan steps).
  - Post-LSTM stages are BATCH-SPLIT across the pair: core c handles the first
    4 batches of its group, core c+4 the last 4 (per-core gather-index input).
  - log_softmax over the span axis is single-pass (scores are bounded ~|4.3|):
    local exp-sums are AllReduce-summed over all 8 cores; the log-Z subtraction
    is folded into the output transpose copies.
  - Host concatenates the outputs of all 8 cores in batch order.

The program is identical on all cores (SPMD); direction and batch assignment
live entirely in the per-core input data (tokens, per-direction weights,
gather indices).
"""

import os
import sys

import numpy as np

for _p in ("/opt/trn_rl_repo",):
    if _p not in sys.path and os.path.isdir(_p):
        sys.path.insert(0, _p)

# bass_utils' traced path hard-imports antenv.axon_hooks, which not every
# image ships; provide an in-process fallback registry so tracing degrades
# gracefully (hook=None -> trace skipped) instead of crashing the kernel.
try:
    import antenv.axon_hooks  # noqa: F401
except ImportError:
    import types

    import antenv

    _ah = types.ModuleType("antenv.axon_hooks")
    _ah._hook = None

    def _set_hook(hook):
        _ah._hook = hook

    def _get_hook():
        return _ah._hook

    _ah.set_axon_ntff_profile_hook = _set_hook
    _ah.get_axon_ntff_profile_hook = _get_hook
    sys.modules["antenv.axon_hooks"] = _ah
    antenv.axon_hooks = _ah

import concourse.bass as bass
import concourse.tile as tile
from concourse import bacc
from concourse import mybir
from concourse.bass_utils import run_bass_kernel_spmd

F32 = mybir.dt.float32
F32R = mybir.dt.float32r
BF16 = mybir.dt.bfloat16
FP8 = mybir.dt.float8e4
I32 = mybir.dt.int32
AF = mybir.ActivationFunctionType
OP = mybir.AluOpType
PM = mybir.MatmulPerfMode
USE_FP8 = bool(int(os.environ.get("KERNEL_FP8", "1")))

# Problem dims (hardcoded per spec)
B, T, V, E, H, C, L = 32, 120, 32000, 256, 512, 20, 10
G4 = 4 * H  # 2048
LN_EPS = 1e-5
NCORES = 8
BL = 8             # batches per scan core
PB = 4             # post-stage batches per core
ROWS = BL * T      # 960
PROWS = PB * T     # 480  (post-stage rows per dir)
CH = 40            # scan steps per AllGather chunk
NCH = T // CH      # 3
PASSR = 2 * PB * CH  # gather rows per pass (fwd+bwd for PB batches, CH steps)

# static span table (matches reference loop order)
_begs, _lens = [], []
for _b in range(T):
    for _l in range(1, min(L, T - _b) + 1):
        _begs.append(_b)
        _lens.append(_l)
BEGS = np.asarray(_begs, np.int32)
LENS = np.asarray(_lens, np.int32)
S = len(_begs)  # 1155
assert S == 1155
SROWS = PB * S     # output rows per core = 4620


def _mspanT() -> np.ndarray:
    """[T, S] span-mean pooling matrix (inv_len folded in)."""
    m = np.zeros((T, S), np.float32)
    for s in range(S):
        m[BEGS[s] : BEGS[s] + LENS[s], s] = 1.0 / LENS[s]
    return m


def _pass_rows(pa: int):
    """Logical rows of gather pass `pa`, in gather order.

    Pass pa covers AllGather chunk pa (scan steps s in [CH*pa, CH*pa+CH)).
    Returns a list of (dir, pb, t) in order: fwd pb0..3 (t ascending), then
    bwd pb0..3.  For fwd, chunk pa holds t in [CH*pa, CH*pa+CH); for bwd,
    scan step s = T-1-t, so chunk pa holds t in [CH*(NCH-1-pa), +CH).
    """
    rows = []
    for d in range(2):
        tlo = CH * pa if d == 0 else CH * (NCH - 1 - pa)
        for pb in range(PB):
            for t in range(tlo, tlo + CH):
                rows.append((d, pb, t))
    return rows


def _gather_idx(is_bwd_core: bool) -> np.ndarray:
    """[2*PROWS, 1] per-core gather table.

    Rows [0:480] = pass A (indices into hs_ag0), [480:960] = pass B (into
    hs_ag1).  Within chunk j's AG output: row = 480*r + (s - 60*j)*8 +
    b_local, r = rank in pair (0=fwd core, 1=bwd core), s = scan step
    (= t for fwd, T-1-t for bwd), b_local = batch index within the group.
    """
    idx = np.empty(2 * PROWS, np.int32)
    boff = PB if is_bwd_core else 0
    for pa in range(NCH):
        for i, (d, pb, t) in enumerate(_pass_rows(pa)):
            s = t if d == 0 else T - 1 - t
            assert s // CH == pa
            idx[pa * PASSR + i] = CH * BL * d + (s - CH * pa) * BL + (boff + pb)
    return idx[:, None]


def _pass_segments(pa: int, c0: int, rows: int):
    """Contiguous (row0, n, dir, col0) copy segments for gathered rows
    [c0, c0+rows) of pass `pa` (col = pb*T + t in the rnnT tiles)."""
    prows = _pass_rows(pa)
    segs = []
    r = c0
    while r < c0 + rows:
        d, pb, t = prows[r]
        n = 1
        while (
            r + n < c0 + rows
            and prows[r + n] == (d, pb, t + n)
        ):
            n += 1
        segs.append((r - c0, n, d, pb * T + t))
        r += n
    return segs


def _r(ap):
    return ap.bitcast(F32R)


def build_program():
    max_stage = int(os.environ.get("KERNEL_MAX_STAGE", "99"))
    scan_steps = int(os.environ.get("KERNEL_SCAN_STEPS", str(T)))
    n_warm = int(os.environ.get("KERNEL_WARM", "2"))
    nc = bacc.Bacc(trn_type="TRN2", num_devices=NCORES)

    # ---- I/O ----
    tok = nc.dram_tensor("tok", [ROWS, 1], I32, kind="ExternalInput")
    gidx = nc.dram_tensor("gidx", [2 * PROWS, 1], I32, kind="ExternalInput")
    emb = nc.dram_tensor("emb", [V, E], F32, kind="ExternalInput")
    w_ihT = nc.dram_tensor("w_ihT", [E, G4], F32R, kind="ExternalInput")
    w_hhT = nc.dram_tensor("w_hhT", [H, G4], F32R, kind="ExternalInput")
    w_hhT8 = nc.dram_tensor("w_hhT8", [H, G4], FP8, kind="ExternalInput")
    gbias = nc.dram_tensor("gbias", [1, G4], F32, kind="ExternalInput")
    lin1_wT = nc.dram_tensor("lin1_wT", [2 * H, H], BF16, kind="ExternalInput")
    lin1_b = nc.dram_tensor("lin1_b", [H, 1], F32, kind="ExternalInput")
    lin2_wT = nc.dram_tensor("lin2_wT", [H, H], F32R, kind="ExternalInput")
    lin2_b = nc.dram_tensor("lin2_b", [1, H], F32, kind="ExternalInput")
    # project-then-pool: spans and label projection are both linear, so we
    # project tokens to C=20 first and pool the tiny [T,C] tile instead of
    # pooling [T,H] and projecting [S,H].  Host precomputes (with LN's
    # gamma folded in): wc = lin2_w.T @ (diag(ln_g) @ label_w),
    # bc = lin2_b @ W', colsum_c = sum_h W'[h,c], label_b' = ln_b@W + b.
    wc = nc.dram_tensor("wc", [H, C], F32R, kind="ExternalInput")
    bc = nc.dram_tensor("bc", [1, C], F32, kind="ExternalInput")
    colsum = nc.dram_tensor("colsum", [1, C], F32, kind="ExternalInput")
    label_b = nc.dram_tensor("label_b", [C, 1], F32, kind="ExternalInput")
    # raw block layout: [128, 36*C] windows + tail rows at cols [36*C:37*C]
    out = nc.dram_tensor("out", [128, 37 * C], F32, kind="ExternalOutput")
    # AllGathered per-core exp-sums; host computes logZ and subtracts it
    lz = nc.dram_tensor("lz", [NCORES, C * 8], F32, kind="ExternalOutput")

    # ---- inline constants (same on every core) ----
    ident_d = nc.inline_tensor(np.eye(128, dtype=np.float32), name="ident")
    mspanT_d = nc.inline_tensor(_mspanT(), name="mspanT")
    ones_d = nc.inline_tensor(np.ones((8, 2), dtype=np.float32), name="ones8")

    with tile.TileContext(nc) as tc:
        # long-lived pools (released at end of build)
        const_p = tc.alloc_tile_pool(name="const", bufs=1)
        whh_p = tc.alloc_tile_pool(name="whh", bufs=1)
        dram_p = tc.alloc_tile_pool(name="dram", bufs=1, space="DRAM")

        ident = const_p.tile([128, 128], F32)
        nc.sync.dma_start(out=ident[:], in_=ident_d[:, :])
        ident_r = const_p.tile([128, 128], F32R)
        nc.gpsimd.dma_start(out=ident_r[:], in_=ident_d[:, :])
        ident_bf = const_p.tile([128, 128], BF16)
        nc.gpsimd.dma_start(out=ident_bf[:], in_=ident_d[:, :])
        ones8 = const_p.tile([8, 2], F32R)
        nc.gpsimd.dma_start(out=ones8[:], in_=ones_d[:, :])

        # LSTM state tiles + zero-fills, hoisted so the memsets run during
        # the initial weight DMAs instead of stalling the scan start.
        st_p = tc.alloc_tile_pool(name="state", bufs=1)
        c_sb = st_p.tile([BL, H], F32)
        nc.vector.memset(c_sb[:], 0.0)
        # h_pad rotates through several buffers so the hs-store DMA (and
        # the AllGather blocking the gpsimd queue mid-scan) never stalls
        # the next step's h write.  bf16: h only feeds the bf16 hs store and
        # the fp8 hT quantization, so nothing downstream loses precision,
        # and the PE transposes run at 1 cyc/row instead of 2.
        NHP = 6
        h_pads = [st_p.tile([32, H], BF16, tag=f"hp{i}", name=f"hp{i}") for i in range(NHP)]
        for i in range(NHP):
            nc.vector.memset(h_pads[i][:], 0.0)
        dum_src = st_p.tile([BL, 512], F32R)
        nc.vector.memset(dum_src[:].bitcast(F32), 0.0)
        hT_all = st_p.tile([128, 128], FP8 if USE_FP8 else F32R)
        if USE_FP8:
            nc.vector.memset(hT_all[:], 0.0)
        else:
            nc.vector.memset(hT_all[:].bitcast(F32), 0.0)

        if USE_FP8:
            # DoubleRow operand layout: per k-pair j, tile [128, 2*G4] holding
            # the two 128-row k-chunks (2j, 2j+1) adjacent on the free axis.
            whh8 = [whh_p.tile([128, 2 * G4], FP8, tag=f"whh8_{j}", name=f"whh8_{j}") for j in range(2)]

            def load_whh():
                for j in range(2):
                    for i in range(2):
                        nc.sync.dma_start(
                            out=whh8[j][:, G4 * i : G4 * (i + 1)],
                            in_=w_hhT8[128 * (2 * j + i) : 128 * (2 * j + i + 1), :],
                        )
        else:
            w_hhT_sb = [whh_p.tile([128, G4], F32R, tag=f"whh{k}", name=f"whh{k}") for k in range(4)]

            def load_whh():
                for k in range(4):
                    nc.sync.dma_start(out=w_hhT_sb[k][:], in_=w_hhT[128 * k : 128 * (k + 1), :])

        xg_dram = dram_p.tile([T, BL, G4], F32R)
        # per-chunk hs (bf16, t-major) + per-chunk AllGather outputs
        hs_dram = [dram_p.tile([CH * BL, H], BF16, name=f"hsd{j}") for j in range(NCH)]
        hs_ag = [dram_p.tile([2 * CH * BL, H], BF16, name=f"hsag{j}") for j in range(NCH)]

        # post-stage constants pool: tiles declared here, but the DMAs are
        # emitted AFTER the scan loop (emit_postw_loads) so they don't
        # delay the pre-phase token/embedding DMAs — they trickle in
        # during the ~500us scan instead.
        postw = tc.alloc_tile_pool(name="postw", bufs=1)
        l1w = [postw.tile([128, H], BF16, tag=f"l1w{k}", name=f"l1w{k}") for k in range(8)]
        l1b = [postw.tile([128, 1], F32, tag=f"l1b{m}", name=f"l1b{m}") for m in range(4)]
        l2w = [postw.tile([128, H], F32R, tag=f"l2w{k}", name=f"l2w{k}") for k in range(4)]
        l2b = postw.tile([T, H], F32)
        wc_sb = [postw.tile([128, C], F32R, tag=f"wc{k}", name=f"wc{k}") for k in range(4)]
        bc_bc = postw.tile([T, C], F32)
        colsum_bc = postw.tile([T, C], F32)
        lblb = postw.tile([C, 1], F32)
        SP = 1160  # S padded (fp32r matmul needs even free dims)
        mspan = postw.tile([T, SP], F32R)
        eps_sb = postw.tile([T, 1], F32)
        partials = postw.tile([C, PB], F32)
        corr_bc = postw.tile([128, C], F32)

        postw_thunks = []
        for k in range(8):
            postw_thunks.append(lambda k=k: nc.sync.dma_start(
                out=l1w[k][:], in_=lin1_wT[128 * k : 128 * (k + 1), :]))
        for m in range(4):
            postw_thunks.append(lambda m=m: nc.sync.dma_start(
                out=l1b[m][:], in_=lin1_b[128 * m : 128 * (m + 1), :]))
        for k in range(4):
            postw_thunks.append(lambda k=k: nc.sync.dma_start(
                out=l2w[k][:], in_=lin2_wT[128 * k : 128 * (k + 1), :]))
        postw_thunks.append(lambda: nc.gpsimd.dma_start(
            out=l2b[:], in_=lin2_b[:, :].to_broadcast([T, H])))
        for k in range(4):
            postw_thunks.append(lambda k=k: nc.sync.dma_start(
                out=wc_sb[k][:], in_=wc[128 * k : 128 * (k + 1), :]))
        postw_thunks.append(lambda: nc.gpsimd.dma_start(
            out=bc_bc[:], in_=bc[:, :].to_broadcast([T, C])))
        postw_thunks.append(lambda: nc.gpsimd.dma_start(
            out=colsum_bc[:], in_=colsum[:, :].to_broadcast([T, C])))
        postw_thunks.append(lambda: nc.sync.dma_start(out=lblb[:], in_=label_b[:, :]))
        postw_thunks.append(lambda: nc.vector.memset(mspan[:].bitcast(F32), 0.0))
        postw_thunks.append(lambda: nc.gpsimd.dma_start(out=mspan[:, :S], in_=mspanT_d[:, :]))
        postw_thunks.append(lambda: nc.vector.memset(eps_sb[:], LN_EPS))

        # big long-lived activations
        big_p = tc.alloc_tile_pool(name="big", bufs=1)
        rnnT = [big_p.tile([128, PROWS], BF16, tag=f"rnnT{j}", name=f"rnnT{j}") for j in range(8)]
        h1T = [big_p.tile([128, PROWS], F32R, tag=f"h1T{m}", name=f"h1T{m}") for m in range(4)]
        scoresT = big_p.tile([C, SROWS], F32)

        # ================= Stage 1: embedding gather + transpose =============
        with tc.tile_pool(name="s1", bufs=3) as s1p, \
             tc.tile_pool(name="s1ps", bufs=4, space="PSUM") as s1ps, \
             tc.tile_pool(name="xt", bufs=1) as xt_p, \
             tc.tile_pool(name="wih", bufs=1) as wih_p:

            xT = [xt_p.tile([128, ROWS], F32R, tag=f"xT{k}", name=f"xT{k}") for k in range(2)]
            w_ihT_sb = [wih_p.tile([128, G4], F32R, tag=f"wih{k}", name=f"wih{k}") for k in range(2)]
            gbias_sb = wih_p.tile([T, G4], F32)
            for k in range(2):
                nc.sync.dma_start(
                    out=w_ihT_sb[k][:], in_=w_ihT[128 * k : 128 * (k + 1), :]
                )
            nc.gpsimd.dma_start(
                out=gbias_sb[:], in_=gbias[:, :].to_broadcast([T, G4])
            )
            load_whh()

            n_full = ROWS // 128          # 7
            tail = ROWS - n_full * 128    # 64
            for r in range(n_full + 1):
                rows = 128 if r < n_full else tail
                idx_sb = s1p.tile([128, 1], I32, tag="idx")
                x_sb = s1p.tile([128, E], F32, tag="x")
                nc.sync.dma_start(
                    out=idx_sb[:rows], in_=tok[r * 128 : r * 128 + rows, :]
                )
                nc.gpsimd.indirect_dma_start(
                    out=x_sb[:rows, :],
                    out_offset=None,
                    in_=emb[:, :],
                    in_offset=bass.IndirectOffsetOnAxis(ap=idx_sb[:rows, :1], axis=0),
                )
                for k in range(2):
                    pt = s1ps.tile([128, 128], F32, tag="pt")
                    nc.tensor.transpose(
                        out=pt[:, :rows],
                        in_=x_sb[:rows, 128 * k : 128 * (k + 1)],
                        identity=ident[:rows, :rows],
                    )
                    nc.vector.tensor_copy(
                        out=xT[k][:, r * 128 : r * 128 + rows], in_=pt[:, :rows]
                    )

            # ================= Stage 2: xg = x @ w_ihT + bias ================
            with tc.tile_pool(name="s2", bufs=3) as s2p, \
                 tc.tile_pool(name="s2ps", bufs=3, space="PSUM") as s2ps:
                for b in range(BL):
                    for n in range(4):
                        ps = s2ps.tile([T, 512], F32, tag="ps")
                        for k in range(2):
                            nc.tensor.matmul(
                                ps[:],
                                lhsT=xT[k][:, b * T : (b + 1) * T],
                                rhs=w_ihT_sb[k][:, 512 * n : 512 * (n + 1)],
                                start=(k == 0),
                                stop=(k == 1),
                            )
                        stg = s2p.tile([T, 512], F32R, tag="stg")
                        nc.vector.tensor_tensor(
                            out=stg[:],
                            in0=ps[:],
                            in1=gbias_sb[:, 512 * n : 512 * (n + 1)],
                            op=OP.add,
                        )
                        nc.sync.dma_start(
                            out=xg_dram[:, b, 512 * n : 512 * (n + 1)], in_=stg[:]
                        )

        # ================= Stage 3: LSTM scan (this core's direction) ========
        # Layout: one PSUM tile per gate pg[32,512]; batch padded 8->32 with
        # zeros so every read row is defined.  The xg contribution is injected
        # by an identity-matmul per gate.  h lives in h_pad [32,512]; one
        # transpose chain produces hT_all [128,128] (f32r) used as the next
        # step's stationary operand.  hs is stored bf16 (cast during DMA) into
        # per-chunk DRAM tiles; each chunk AllGathers with the pair core as
        # soon as its last step is stored, overlapping the remaining scan.
        s5p = tc.alloc_tile_pool(name="s5", bufs=3)
        s5ps = tc.alloc_tile_pool(name="s5ps", bufs=1, space="PSUM")
        # early gather passes (all but the last) run inside the scan, 12+
        # steps after their chunk's AllGather is issued
        GATHER_SCHED = {}
        for _pa in range(NCH - 1):
            for _cch in range((PASSR + 127) // 128):
                GATHER_SCHED[CH * (_pa + 1) + 12 + 4 * _cch] = (_pa, _cch)

        def emit_gather_chunk(pa: int, cch: int):
            """Gather + transpose 128 rows of pass `pa` into rnnT."""
            base = pa * PASSR + cch * 128
            rows = min(128, PASSR - cch * 128)
            idx_sb = s5p.tile([128, 1], I32, tag="gidx")
            nc.sync.dma_start(out=idx_sb[:rows], in_=gidx[base : base + rows, :])
            t_sb = s5p.tile([128, H], BF16, tag="hrows")
            nc.gpsimd.indirect_dma_start(
                out=t_sb[:rows, :],
                out_offset=None,
                in_=hs_ag[pa][:, :],
                in_offset=bass.IndirectOffsetOnAxis(ap=idx_sb[:rows, :1], axis=0),
            )
            segs = _pass_segments(pa, cch * 128, rows)
            for k in range(4):
                pt = s5ps.tile([128, 128], BF16, tag="pt")
                nc.tensor.transpose(
                    out=pt[:, :rows], in_=t_sb[:rows, 128 * k : 128 * (k + 1)],
                    identity=ident_bf[:rows, :rows],
                )
                for si, (row0, n, dd, col0) in enumerate(segs):
                    if (k + si) % 2 == 0:
                        nc.vector.tensor_copy(
                            out=rnnT[dd * 4 + k][:, col0 : col0 + n],
                            in_=pt[:, row0 : row0 + n],
                        )
                    else:
                        nc.scalar.copy(
                            out=rnnT[dd * 4 + k][:, col0 : col0 + n],
                            in_=pt[:, row0 : row0 + n],
                        )

        with tc.tile_pool(name="xg", bufs=3) as xg_p, \
             tc.tile_pool(name="gt", bufs=3) as gt_p, \
             tc.tile_pool(name="gps", bufs=2, space="PSUM") as gps, \
             tc.tile_pool(name="tps", bufs=1, space="PSUM") as tps, \
             tc.tile_pool(name="dps", bufs=1, space="PSUM") as dps:

            pdum = dps.tile([32, 512], F32, name="pdum")

            # gate order: gg, i, f, o — the tanh chain starts as early as possible
            GATE_ORDER = (2, 0, 1, 3)
            n_steps = scan_steps if max_stage >= 3 else 1

            xg_tiles = {}

            def load_xg(s):
                xg_tiles[s] = xg_p.tile([BL, G4], F32R, tag="xg", name="xg_s")
                nc.sync.dma_start(out=xg_tiles[s][:], in_=xg_dram[s, :, :])

            pg_tiles = {}

            def emit_injects(s):
                # xg inject (start=True) into each gate's PSUM bank.  These
                # only depend on the xg DMA and the bank's previous reader,
                # so they run on the PE during the PREVIOUS step's tail —
                # useful work instead of idle/filler.  The o bank is
                # double-buffered since its step-s reader (sig_o) is the
                # last thing to complete in step s.
                xg_s = xg_tiles.pop(s)
                pgs = {}
                for n in GATE_ORDER:
                    pg = gps.tile(
                        [32, 512], F32, tag=f"pg{n}",
                        bufs=2 if n == 3 else 1, name=f"pg{n}",
                    )
                    nc.tensor.matmul(
                        pg[:],
                        lhsT=ident_r[:BL, :32],
                        rhs=xg_s[:, 512 * n : 512 * (n + 1)],
                        start=True,
                        stop=False,
                    )
                    pgs[n] = pg
                pg_tiles[s] = pgs

            def emit_filler(cnt):
                # dependency-free fat fillers: keep the PE busy through the
                # tail so the HAM activity monitor grants the 2.4GHz clock;
                # they run while the PE would otherwise idle waiting for the
                # hT cast, so they are off the critical path
                for _ in range(cnt):
                    nc.tensor.matmul(
                        pdum[:],
                        lhsT=ident_r[:BL, :32],
                        rhs=dum_src[:],
                        start=True,
                        stop=True,
                    )

            load_xg(0)
            if n_steps > 1:
                load_xg(1)
            emit_injects(0)

            for s in range(n_steps):
                h_pad = h_pads[s % NHP]
                if s + 2 < n_steps:
                    load_xg(s + 2)

                acts = {}
                for gi, n in enumerate(GATE_ORDER):
                    pg = pg_tiles[s][n]
                    if USE_FP8:
                        for j in range(2):
                            nc.tensor.matmul(
                                pg[:],
                                lhsT=hT_all[:, 64 * j : 64 * (j + 1)].rearrange(
                                    "p (i b) -> p i b", i=2
                                ),
                                rhs=whh8[j][:, :].rearrange(
                                    "p (i c) -> p i c", i=2
                                )[:, :, 512 * n : 512 * (n + 1)],
                                start=False,
                                stop=(j == 1),
                                perf_mode=PM.DoubleRow,
                            )
                    else:
                        for k in range(4):
                            nc.tensor.matmul(
                                pg[:],
                                lhsT=hT_all[:, 32 * k : 32 * (k + 1)],
                                rhs=w_hhT_sb[k][:, 512 * n : 512 * (n + 1)],
                                start=False,
                                stop=(k == 3),
                            )
                    a_sb = gt_p.tile([BL, 512], F32, tag=f"a{n}")
                    if n in (1, 3):  # f, o: halved so the c/h chains pipeline
                        for hf in (0, 1):
                            sl = slice(256 * hf, 256 * (hf + 1))
                            nc.scalar.activation(
                                out=a_sb[:, sl], in_=pg[:BL, sl], func=AF.Sigmoid
                            )
                            if n == 3:  # tanh(c) then h = sig_o * tanh(c)
                                nc.scalar.activation(
                                    out=tch[:, sl], in_=c_sb[:, sl], func=AF.Tanh
                                )
                                nc.vector.tensor_tensor(
                                    out=h_pad[:BL, sl], in0=a_sb[:, sl],
                                    in1=tch[:, sl], op=OP.mult,
                                )
                    else:
                        nc.scalar.activation(
                            out=a_sb[:],
                            in_=pg[:BL, :],
                            func=AF.Tanh if n == 2 else AF.Sigmoid,
                        )
                    acts[n] = a_sb
                    if n == 0:  # after i (2nd group): t1 = sig_i * tanh_gg
                        t1 = gt_p.tile([BL, H], F32, tag="t1")
                        nc.vector.tensor_tensor(
                            out=t1[:], in0=acts[0][:], in1=acts[2][:], op=OP.mult
                        )
                    elif n == 1:  # after f (3rd group): c = c*f + t1
                        tch = gt_p.tile([BL, H], F32, tag="tch")
                        for hf in (0, 1):
                            sl = slice(256 * hf, 256 * (hf + 1))
                            nc.vector.tensor_tensor(
                                out=c_sb[:, sl], in0=c_sb[:, sl], in1=acts[1][:, sl],
                                op=OP.mult,
                            )
                            nc.vector.tensor_tensor(
                                out=c_sb[:, sl], in0=c_sb[:, sl], in1=t1[:, sl],
                                op=OP.add,
                            )

                # next step's injects fill the PE during this step's tail
                if s + 1 < n_steps:
                    emit_injects(s + 1)

                nc.gpsimd.dma_start(
                    out=hs_dram[s // CH][(s % CH) * BL : (s % CH + 1) * BL, :],
                    in_=h_pad[:BL, :],
                )

                pt_all = tps.tile([128, 128], BF16, tag="pt")
                for q in range(4):
                    nc.tensor.transpose(
                        out=pt_all[:, 32 * q : 32 * (q + 1)],
                        in_=h_pad[:, 128 * q : 128 * (q + 1)],
                        identity=ident_bf[:32, :32],
                    )
                # split copy: the next step's first gate MMs only need the
                # low half of hT, so let them start as soon as q0/q1 land
                nc.vector.tensor_copy(out=hT_all[:, 0:64], in_=pt_all[:, 0:64])
                nc.vector.tensor_copy(out=hT_all[:, 64:128], in_=pt_all[:, 64:128])

                emit_filler(n_warm)

                # trickle the post-stage constant loads into the scan, a
                # couple per step, so they never head-of-line-block the
                # per-step xg loads
                if s >= 1:
                    for _ in range(2):
                        if postw_thunks:
                            postw_thunks.pop(0)()

                # ====== Stage 4: chunked AllGather with the pair core =======
                if max_stage >= 4 and (s + 1) % CH == 0 and s + 1 <= NCH * CH:
                    j = (s + 1) // CH - 1
                    nc.gpsimd.collective_compute(
                        "AllGather",
                        OP.bypass,
                        replica_groups=[[0, 4], [1, 5], [2, 6], [3, 7]],
                        ins=[hs_dram[j][:].opt()],
                        outs=[hs_ag[j][:].opt()],
                    )

                # ====== Stage 5a: early-pass gathers overlap the scan =======
                if max_stage >= 5 and scan_steps == T and s in GATHER_SCHED:
                    emit_gather_chunk(*GATHER_SCHED[s])

        while postw_thunks:
            postw_thunks.pop(0)()

        # ============ Stage 5b: final-pass gathers (need the last AG) ====
        if max_stage >= 5:
            if scan_steps != T:
                for pa in range(NCH - 1):
                    for cch in range((PASSR + 127) // 128):
                        emit_gather_chunk(pa, cch)
            for cch in range((PASSR + 127) // 128):
                emit_gather_chunk(NCH - 1, cch)
        s5p.release()
        s5ps.release()

        # ================= Stage 6: h1T = relu(lin1) ====================
        with tc.tile_pool(name="s6ps", bufs=2, space="PSUM") as s6ps:
            for m in range(4 if max_stage >= 6 else 0):
                ph = s6ps.tile([128, PROWS], F32, tag="ph")
                for k in range(8):
                    nc.tensor.matmul(
                        ph[:],
                        lhsT=l1w[k][:, 128 * m : 128 * (m + 1)],
                        rhs=rnnT[k][:],
                        start=(k == 0),
                        stop=(k == 7),
                    )
                nc.scalar.activation(
                    out=h1T[m][:],
                    in_=ph[:],
                    func=AF.Relu,
                    bias=l1b[m][:],
                )

        # ============ Stages 7-9: per-batch lin2+LN+span+label ==========
        with tc.tile_pool(name="s7", bufs=3) as s7p, \
             tc.tile_pool(name="s7ps", bufs=2, space="PSUM") as s7ps:

            SCH = ((0, 512), (512, 512), (1024, 136))
            for b in range(PB if max_stage >= 7 else 0):
                ph2 = s7ps.tile([T, 512], F32, tag="ph2")
                for k in range(4):
                    nc.tensor.matmul(
                        ph2[:],
                        lhsT=h1T[k][:, b * T : (b + 1) * T],
                        rhs=l2w[k][:],
                        start=(k == 0),
                        stop=(k == 3),
                    )
                trh = s7p.tile([T, H], F32, tag="trh")
                nc.vector.tensor_tensor(
                    out=trh[:], in0=ph2[:], in1=l2b[:], op=OP.add,
                )
                # LayerNorm stats over H
                stats = s7p.tile([T, 6], F32, tag="stats")
                nc.vector.bn_stats(out=stats[:], in_=trh[:])
                mv = s7p.tile([T, 2], F32, tag="mv")
                nc.vector.bn_aggr(out=mv[:], in_=stats[:])
                sd = s7p.tile([T, 1], F32, tag="sd")
                nc.scalar.activation(
                    out=sd[:], in_=mv[:, 1:2], func=AF.Sqrt, bias=eps_sb[:]
                )
                rstd = s7p.tile([T, 1], F32, tag="rstd")
                nc.vector.reciprocal(out=rstd[:], in_=sd[:])
                # beta = -mu * rstd  (per-token scalar)
                beta = s7p.tile([T, 1], F32, tag="beta")
                nc.vector.tensor_scalar(
                    out=beta[:], in0=mv[:, 0:1],
                    scalar1=rstd[:], scalar2=-1.0,
                    op0=OP.mult, op1=OP.mult,
                )
                # project tokens to classes FIRST (tiny N=20 matmuls):
                # phC = (h1 @ wc) + bc;  trC = rstd*phC + beta*colsum
                # equals LN(h) @ W' per token — pooling commutes with both
                phc = s7ps.tile([T, C], F32, tag="phc")
                for k in range(4):
                    nc.tensor.matmul(
                        phc[:],
                        lhsT=h1T[k][:, b * T : (b + 1) * T],
                        rhs=wc_sb[k][:],
                        start=(k == 0),
                        stop=(k == 3),
                    )
                u1 = s7p.tile([T, C], F32, tag="u1")
                nc.vector.tensor_tensor(
                    out=u1[:], in0=phc[:], in1=bc_bc[:], op=OP.add,
                )
                nc.vector.tensor_scalar(
                    out=u1[:], in0=u1[:],
                    scalar1=rstd[:], scalar2=1.0,
                    op0=OP.mult, op1=OP.mult,
                )
                trc = s7p.tile([T, C], F32R, tag="trc")
                nc.vector.scalar_tensor_tensor(
                    out=trc[:], in0=colsum_bc[:], scalar=beta[:],
                    in1=u1[:], op0=OP.mult, op1=OP.add,
                )
                # span pooling directly in class space: [20, S] chunks
                for n0, nw in SCH:
                    psc = s7ps.tile([C, 512], F32, tag="psc")
                    nc.tensor.matmul(
                        psc[:, :nw],
                        lhsT=trc[:],
                        rhs=mspan[:, n0 : n0 + nw],
                        start=True,
                        stop=True,
                    )
                    w_real = min(nw, S - n0)
                    nc.scalar.activation(
                        out=scoresT[:, b * S + n0 : b * S + n0 + w_real],
                        in_=psc[:, :w_real],
                        func=AF.Identity,
                        bias=lblb[:],
                    )
                # single-pass softmax: exp-sum of this batch's scores
                expc = s7p.tile([C, S], F32, tag="expc")
                nc.scalar.activation(
                    out=expc[:], in_=scoresT[:, b * S : (b + 1) * S],
                    func=AF.Exp,
                    accum_out=partials[:, b : b + 1],
                )

        # ==== Stages 10/11: log-sum-exp AllReduce overlapped with ========
        # ==== the raw-score output transposes; logZ subtracted last ======
        with tc.tile_pool(name="s10", bufs=1) as s10p, \
             tc.tile_pool(name="outp", bufs=1) as out_p, \
             tc.tile_pool(name="ops", bufs=4, space="PSUM") as ops:
            cc_in = dram_p.tile([C, 8], F32)
            cc_ag = dram_p.tile([NCORES * C, 8], F32)
            logz_dram = dram_p.tile([1, C], F32)
            RG = [[0, 1, 2, 3, 4, 5, 6, 7]]

            if max_stage < 10:
                nc.vector.memset(scoresT[:, :4], 0.0)  # keep scoresT written
            lsum = s10p.tile([C, 1], F32)
            nc.vector.tensor_reduce(
                out=lsum[:], in_=partials[:], axis=mybir.AxisListType.X,
                op=OP.add,
            )
            stg8 = s10p.tile([C, 8], F32)
            nc.vector.tensor_copy(out=stg8[:], in_=lsum[:].to_broadcast([C, 8]))
            nc.sync.dma_start(out=cc_in[:, :], in_=stg8[:])
            # AllGather the 8 per-core exp-sums, reduce locally on the PE
            # (consistently ~2-3x faster than the tiny AllReduce here)
            nc.gpsimd.collective_compute(
                "AllGather", OP.bypass, replica_groups=RG,
                ins=[cc_in[:].opt()], outs=[cc_ag[:].opt()],
            )
            sb8 = s10p.tile([8, C * 8], F32)
            nc.gpsimd.dma_start(
                out=sb8[:],
                in_=cc_ag[:, :].rearrange("(r c) w -> r (c w)", r=NCORES),
            )
            nc.sync.dma_start(out=lz[:, :], in_=sb8[:])

            # raw-score transposes run while the AllReduce is in flight
            n_full = SROWS // 128  # 36
            tail = SROWS - n_full * 128  # 12
            out_sb = out_p.tile([128, n_full * C], F32)
            out_tail = out_p.tile([tail, C], F32)
            for m in range(n_full + 1):
                cw = 128 if m < n_full else tail
                pto = ops.tile([128, C], F32, tag="pto")
                nc.tensor.transpose(
                    out=pto[:cw, :],
                    in_=scoresT[:, 128 * m : 128 * m + cw],
                    identity=ident[:C, :C],
                )
                if m < n_full:
                    if m % 2 == 0:
                        nc.vector.tensor_copy(
                            out=out_sb[:, m * C : (m + 1) * C], in_=pto[:, :]
                        )
                    else:
                        nc.scalar.copy(
                            out=out_sb[:, m * C : (m + 1) * C], in_=pto[:, :]
                        )
                else:
                    nc.vector.tensor_copy(
                        out=out_tail[:], in_=pto[:cw, :]
                    )

            nc.sync.dma_start(out=out[:, : n_full * C], in_=out_sb[:])
            nc.sync.dma_start(
                out=out[: tail, n_full * C :], in_=out_tail[:]
            )

        big_p.release()
        postw.release()
        st_p.release()
        whh_p.release()

        const_p.release()
        dram_p.release()

    nc.finalize()
    return nc


_PROGRAM = None


def _get_program():
    global _PROGRAM
    if _PROGRAM is None:
        _PROGRAM = build_program()
    return _PROGRAM


def _in_maps(inputs: dict) -> list[dict]:
    import ml_dtypes

    tokens = np.ascontiguousarray(np.asarray(inputs["tokens"], np.int32))
    emb = np.ascontiguousarray(np.asarray(inputs["emb"], np.float32))
    shared = {
        "emb": emb,
        "lin1_wT": np.ascontiguousarray(
            np.asarray(inputs["lin1_w"], np.float32).T.astype(ml_dtypes.bfloat16)
        ),
        "lin1_b": np.ascontiguousarray(
            np.asarray(inputs["lin1_b"], np.float32)[:, None]
        ),
        "lin2_wT": np.ascontiguousarray(np.asarray(inputs["lin2_w"], np.float32).T),
        "lin2_b": np.ascontiguousarray(
            np.asarray(inputs["lin2_b"], np.float32)[None, :]
        ),
    }
    # project-then-pool folds (all exact linear algebra):
    #   W' = diag(ln_g) @ label_w          (LN gamma into label weights)
    #   wc = lin2_w.T @ W'                 (lin2 into the class projection)
    #   bc = lin2_b @ W'                   (lin2 bias contribution pre-LN-scale)
    #   colsum_c = sum_h W'[h,c]           (for the -mu*rstd rank-1 term)
    #   label_b' = ln_b @ label_w + label_b  (post-LN constants)
    ln_g_v = np.asarray(inputs["ln_g"], np.float32)
    ln_b_v = np.asarray(inputs["ln_b"], np.float32)
    lw = np.asarray(inputs["label_w"], np.float32)
    lb = np.asarray(inputs["label_b"], np.float32)
    w_eff = lw * ln_g_v[:, None]
    lin2_wT_np = np.asarray(inputs["lin2_w"], np.float32).T
    shared["wc"] = np.ascontiguousarray(lin2_wT_np @ w_eff)
    shared["bc"] = np.ascontiguousarray(
        (np.asarray(inputs["lin2_b"], np.float32) @ w_eff)[None, :]
    )
    shared["colsum"] = np.ascontiguousarray(w_eff.sum(axis=0)[None, :])
    shared["label_b"] = np.ascontiguousarray((ln_b_v @ lw + lb)[:, None])
    fp8_np = mybir.dt.np(FP8)
    per_dir = {}
    for d, sfx in ((0, "f"), (1, "b")):
        whht = np.asarray(inputs[f"w_hh_{sfx}"], np.float32).T
        per_dir[d] = {
            "w_ihT": np.ascontiguousarray(
                np.asarray(inputs[f"w_ih_{sfx}"], np.float32).T
            ),
            "w_hhT": np.ascontiguousarray(whht),
            "w_hhT8": np.ascontiguousarray(whht.astype(fp8_np)),
            "gbias": np.ascontiguousarray(
                (
                    np.asarray(inputs[f"b_ih_{sfx}"], np.float32)
                    + np.asarray(inputs[f"b_hh_{sfx}"], np.float32)
                )[None, :]
            ),
        }
    gidx_f = _gather_idx(False)
    gidx_b = _gather_idx(True)
    maps = []
    for core in range(NCORES):
        d = core // 4
        g = core % 4
        tk = tokens[g * BL : (g + 1) * BL]  # [8, 120]
        if d == 1:
            tk = tk[:, ::-1]
        m = dict(shared)
        m.update(per_dir[d])
        m["tok"] = np.ascontiguousarray(tk.reshape(-1)[:, None])
        m["gidx"] = np.ascontiguousarray(gidx_b if d == 1 else gidx_f)
        maps.append(m)
    return maps


def kernel(**inputs) -> np.ndarray:
    nc = _get_program()
    res = run_bass_kernel_spmd(
        nc,
        _in_maps(inputs),
        core_ids=list(range(NCORES)),
        trace=bool(int(os.environ.get("KERNEL_TRACE", "0"))),
    )
    kernel.last_results = res
    # batch order: g-major, fwd core (first 4 batches) then bwd core (last 4)
    n_full = SROWS // 128  # 36
    tail = SROWS - n_full * 128
    outs = []
    for g in range(4):
        for core in (g, g + 4):
            raw = res.results[core]["out"]
            # logZ = log of the 8 AllGathered per-core exp-sums
            logz = np.log(
                res.results[core]["lz"][:, ::8].astype(np.float32).sum(axis=0)
            ).astype(np.float32)
            body = raw[:, : n_full * C].reshape(128, n_full, C)
            body = np.transpose(body, (1, 0, 2)).reshape(n_full * 128, C)
            outs.append(body - logz)
            outs.append(raw[:tail, n_full * C :] - logz)
    return np.ascontiguousarray(np.concatenate(outs, axis=0))

